# revision 15
# baseline (speedup 1.0000x reference)
"""VQ codebook assignment + nearest upsample on 8 NeuronCores.

Problem (per domain): given features f [B=4, C=256, H=64, W=128] and
centroids c [K=19, C=256], compute argmin_k ||f[b,:,h,w] - c_k||^2 and
nearest-upsample the [64,128] index map to [512,1024] (8x per axis).
Two domains (cross-assigned centroids) x 4 batches = 8 cores, one
batch-image per core, no cross-core communication.

Design (fp16 matmul + int32 fixed-point scores + packed argmin);
measured ~38 us max-core / ~35.6 us mean vs the 64-76 us fp32 v1:

  * Features/centroids rounded to fp16 on the host: 1 cycle/row on the
    PE (fp32 is 4) and 4.2 MB/core of input DMA (half of fp32).
    Measured flip rate vs the fp32 reference: 0.04% of pixels ->
    rel_err 1.50e-2, under the 2e-2 gate (bf16 fails at 3.8e-2; the
    error is dominated by fp16 input rounding, deterministic for the
    fixed test inputs).
  * Centroids are pre-scaled by 256 (exact in fp16, a power of two), so
    fp32 PSUM scores are 256*(f.c_k). A bit-exact ScalarE Copy (the
    non-LUT Activation path) converts them to int32 score units; all
    downstream arithmetic is exact integer math. Score quantization
    error (1/512 in feature units) is ~3x below the fp16 input error.
  * -|c_k|^2/2 bias is folded into a host-built int32 "bias-iota"
    table: B = -32*score + (-32*bq_k + k), computed by one DVE
    scalar_tensor_tensor, then ONE min-reduce over k and (B & 31)
    recovers k. Ties pick the smaller k = jnp.argmin first-match
    semantics, exactly. Padding k's (19..31) get +2^30 so they never
    win. This replaces the 5-op max/is_ge/select/min argmax chain --
    DVE reduces and comparisons run at 1x (only some ALU ops get the
    2-byte 2x mode), so fewer + cheaper ops win.
  * The K-partition -> pixel-partition transpose that cost the fp32 v1
    64 PE passes is ONE DVE 32x32 StreamTranspose per 2048-px
    superblock, reading the packed int32 score tile. The host
    pre-permutes feature pixels into (sb, cch, h%16, w%32) tile order
    so the block-transposed layout lands directly as idxv[w, h]: the
    matmul for chunk cch writes PSUM partitions 32cch..+32 (PE PSUM
    writes only allow partition bases {0,32,64} -> two [64,512] PSUM
    tiles per superblock), and after the 32x32 block transpose
    partition = w, column-block = h. The last superblock's chain is
    column-split in two so the drain pipelines.
  * DMA: each HWDGE queue (SP/ACT) sustains ~150-190 GB/s only when
    pieces queue back-to-back with ~4KB-per-partition packets (the
    per-queue window is ~4 packets in flight at ~23 GB/s per DMA
    engine; the gpsimd SWDGE queue dispatches ~4x slower, ~24 GB/s, and
    only carries the tiny bias table). Input pieces arrive in
    superblock order on alternating queues (~290 GB/s aggregate,
    ~14 us); sb3 is split so the last piece is small.
  * Upsample tail per h-half: DVE 32x32 transpose + block copies build
    idxT[h, w], a broadcast copy replicates 8x in x converting to int8
    (split DVE+ACT mid-kernel, DVE-only for the drain-critical half),
    and stores write [*, 8y, 1024] with a stride-0 source loop for the
    8x y-replication (1KB runs; the final store splits across both
    HWDGE queues). int8 mask out (0.5 MB); the host upcasts to int32.

Remaining wall-clock anatomy (per core): ~8 us fixed prologue (runtime
handshake + DGE ring config + barriers), ~14 us input DMA, ~8 us
drain (sem-prop + last superblock compute + replicate + store), ~3 us
epilogue. PE ~10 us and DVE ~12 us active both hide under the DMA.
"""

import numpy as np

import concourse.bass as bass
import concourse.mybir as mybir
import concourse.tile as tile
from concourse import bacc
from concourse.bass import ds
from concourse.bass_utils import run_bass_kernel_spmd

F32 = mybir.dt.float32
F16 = mybir.dt.float16
I32 = mybir.dt.int32
I16 = mybir.dt.int16
I8 = mybir.dt.int8

B = 4
C = 256
H, W = 64, 128
K = 19
KP = 32               # K padded to a 32x32 transpose block
HL, WL = 512, 1024
NPIX = H * W          # 8192
SB = 4                # superblocks (2048 px each)
SBPIX = NPIX // SB
CH = 512              # matmul moving chunk (pixels)
NCH = SBPIX // CH     # chunks per superblock: 4
NJ = CH // KP         # 32-col blocks per score tile: 16
UP = HL // H          # 8x upsample
SC = 256.0            # centroid pre-scale -> int16 score units
FWC = KP + NPIX       # fw columns: [w | pixels]

_NC_CACHE = None


def _build_nc():
    nc = bacc.Bacc("TRN2", target_bir_lowering=False, debug=False)

    fw_in = nc.dram_tensor("fw", [C, FWC], F16, kind="ExternalInput")
    bi_in = nc.dram_tensor("biasiota", [128, KP], I32, kind="ExternalInput")
    mask_out = nc.dram_tensor("mask", [HL, WL], I8, kind="ExternalOutput")

    fwv = fw_in.ap().rearrange("(a p) n -> a p n", a=2)       # [2, 128, FWC]
    outv = mask_out.ap().rearrange("(h y) x -> h y x", y=UP)  # [64, 8, 1024]

    with tile.TileContext(nc) as tc:
        with (
            tc.tile_pool(name="persist", bufs=1) as pp,
            tc.tile_pool(name="work", bufs=2) as wp,
            tc.tile_pool(name="psA", bufs=3, space="PSUM") as psA,
        ):
            fw0 = pp.tile([128, FWC], F16, tag="fw0")
            fw1 = pp.tile([128, FWC], F16, tag="fw1")
            bi32 = pp.tile([128, KP], I32, tag="bi32")
            idxv = pp.tile([128, H], I32, tag="idxv")       # [w, h]
            tmp16 = pp.tile([128, H], I32, tag="tmp16")     # block-transposed
            idxT = pp.tile([H, W], I32, tag="idxT")         # [h, w]
            rep = pp.tile([H, WL], I8, tag="rep")           # x-replicated

            # --- input loads. Both HWDGE queues (SP/ACT) sustain
            # ~150-190 GB/s each when pieces queue back-to-back (the SWDGE
            # gpsimd queue only manages ~24 GB/s -- it gets just the tiny
            # bias load). Pieces arrive in superblock order on alternating
            # queues; sb3 is split so its last piece is small (short
            # drain). ---
            nc.gpsimd.dma_start(bi32, bi_in[:, :])
            pieces = [
                ds(0, KP + SBPIX),                           # w + sb0
                ds(KP + SBPIX, SBPIX),                       # sb1
                ds(KP + 2 * SBPIX, SBPIX),                   # sb2
                ds(KP + 3 * SBPIX, 3 * SBPIX // 4),          # sb3 front
                ds(KP + 3 * SBPIX + 3 * SBPIX // 4, SBPIX // 4),  # sb3 tail
            ]
            for pi, sl in enumerate(pieces):
                for half in range(2):
                    dst = fw0 if half == 0 else fw1
                    eng = nc.sync if (pi + half) % 2 == 0 else nc.scalar
                    eng.dma_start(dst[:, sl], fwv[half, :, sl])

            bi_b = bi32.rearrange("p (o k) -> p o k", o=1).to_broadcast(
                [128, NJ, KP]
            )

            # --- per-superblock: 8 matmuls -> int16 scores -> 32x32 block
            # transpose -> packed argmin over k ---
            for sb in range(SB):
                psa = psA.tile([64, CH], F32, tag="psa")
                psb = psA.tile([64, CH], F32, tag="psb")
                pst = [psa, psb]
                for cch in range(NCH):
                    colsl = ds(KP + sb * SBPIX + cch * CH, CH)
                    ps = pst[cch // 2]
                    psl = ds(32 * (cch % 2), 32)
                    nc.tensor.matmul(
                        ps[psl, :], fw0[:, 0:KP], fw0[:, colsl],
                        start=True, stop=False,
                    )
                    nc.tensor.matmul(
                        ps[psl, :], fw1[:, 0:KP], fw1[:, colsl],
                        start=False, stop=True,
                    )
                # bit-exact ScalarE Copy: fp32 PSUM -> int32 (RNE)
                St = wp.tile([128, CH], I32, tag="St")
                nc.scalar.copy(St[ds(0, 64), :], pst[0])
                nc.scalar.copy(St[ds(64, 64), :], pst[1])
                # DVE 32x32 block transpose -> partition=w, col-block=h,
                # then B = -32*score + (-32*bq_k + k); min over k; k = B&31.
                # The last superblock is column-split in two so its chain
                # pipelines during the drain.
                T = wp.tile([128, CH], I32, tag="T")
                Bt = wp.tile([128, CH], I32, tag="Bt")
                Bm = wp.tile([128, NJ], I32, tag="Bm")
                nsp = 2 if sb == SB - 1 else 1
                cw = CH // nsp
                for cs in range(nsp):
                    csl = ds(cs * cw, cw)
                    nc.vector.transpose(T[:, csl], St[:, csl])
                    nc.vector.scalar_tensor_tensor(
                        Bt[:, csl].rearrange("p (j k) -> p j k", k=KP),
                        T[:, csl].rearrange("p (j k) -> p j k", k=KP),
                        -32, bi32.rearrange("p (o k) -> p o k", o=1)
                        .to_broadcast([128, cw // KP, KP]),
                        op0=mybir.AluOpType.mult, op1=mybir.AluOpType.add,
                    )
                    bsl = ds(cs * (cw // KP), cw // KP)
                    nc.vector.tensor_reduce(
                        Bm[:, bsl],
                        Bt[:, csl].rearrange("p (j k) -> p j k", k=KP),
                        axis=mybir.AxisListType.X, op=mybir.AluOpType.min,
                    )
                    nc.vector.tensor_scalar(
                        idxv[:, ds(sb * NJ + cs * (cw // KP), cw // KP)],
                        Bm[:, bsl], 31, None,
                        op0=mybir.AluOpType.bitwise_and,
                    )

                # --- tail, overlapped: after each half of the superblocks,
                # emit that h-half (transpose, x8-replicate, store) ---
                if sb % (SB // 2) != SB // 2 - 1:
                    continue
                hh = sb // (SB // 2)           # 0 or 1
                hsl = ds(hh * H // 2, H // 2)  # 32 h columns
                psl = ds(hh * 32, 32)          # matching partition rows
                # the first tail is ready mid-kernel: emit it at high
                # priority so the static scheduler doesn't defer it into
                # the drain behind sb2/sb3's DVE chains (measured +4us)
                prio = tc.high_priority() if hh == 0 else None
                if prio is not None:
                    prio.__enter__()
                nc.vector.transpose(tmp16[:, hsl], idxv[:, hsl])
                for i in range(W // 32):
                    eng = nc.vector if (hh == 0 or i < 2) else nc.scalar
                    eng_copy = (
                        nc.vector.tensor_copy if eng is nc.vector
                        else nc.scalar.copy
                    )
                    eng_copy(
                        idxT[psl, ds(32 * i, 32)],
                        tmp16[ds(32 * i, 32), hsl],
                    )
                # 8x replicate along x, int8 out. Mid-kernel half splits
                # across DVE+ScalarE; the final half runs entirely on DVE
                # (ScalarE showed multi-us scheduling gaps at the drain).
                repv = rep[psl].rearrange("p (w x) -> p w x", w=W)
                for wh in range(2):
                    idxT_b = idxT[psl, ds(wh * W // 2, W // 2)].rearrange(
                        "p (w o) -> p w o", o=1
                    ).to_broadcast([32, W // 2, UP])
                    half = repv[:, ds(wh * W // 2, W // 2)]
                    if wh == 0:
                        nc.vector.tensor_copy(half, idxT_b)
                    else:
                        nc.scalar.copy(half, idxT_b)
                # single store-DMA per half; stride-0 source loop re-reads
                # each 1KB SBUF row 8x for the y-replication
                # stores: hh0 whole on SP (overlaps compute); the final
                # one splits 2x2 across both HWDGE queues so each queue
                # pipelines two entries (shorter drain)
                if hh == 0:
                    splits = ((nc.sync, 0, 32),)
                else:
                    splits = (
                        (nc.sync, 0, 8), (nc.scalar, 8, 8),
                        (nc.sync, 16, 8), (nc.scalar, 24, 8),
                    )
                for eng, p0, np_ in splits:
                    pssl = ds(hh * 32 + p0, np_)
                    srcap = rep[pssl].rearrange(
                        "p (o x) -> p o x", o=1
                    ).to_broadcast([np_, UP, WL])
                    eng.dma_start(outv[pssl], srcap)
                if prio is not None:
                    prio.__exit__(None, None, None)

    nc.compile()
    return nc


def _prep_domain(feature, centroid):
    """Per-core inputs for one domain: 4 batches against one centroid set."""
    c = np.asarray(centroid, dtype=np.float64)                  # [K, C]
    w16 = c.T.astype(np.float16)                                # [C, K]
    wsc = (w16.astype(np.float32) * SC).astype(np.float16)      # exact x2^8
    wpad = np.zeros((C, KP), dtype=np.float16)
    wpad[:, :K] = wsc
    c2 = np.sum(c * c, axis=1)                                  # [K]
    bq = np.rint(SC * (c2.mean() - c2) / 2.0).astype(np.int64)
    biasiota = np.full(KP, 2**30, dtype=np.int64)
    biasiota[:K] = -32 * bq + np.arange(K)
    biasiota = np.ascontiguousarray(
        np.tile(biasiota[None, :], (128, 1)), dtype=np.int32
    )
    maps = []
    for b in range(B):
        f16 = np.asarray(feature[b], dtype=np.float32).astype(np.float16)
        # pixel permutation: image (h, w) -> chunk order (sb, cch, h%16, w%32)
        fp = (
            f16.reshape(C, SB, 16, W // 32, 32)
            .transpose(0, 1, 3, 2, 4)
            .reshape(C, NPIX)
        )
        fw = np.ascontiguousarray(np.concatenate([wpad, fp], axis=1))
        maps.append({"fw": fw, "biasiota": biasiota})
    return maps


def kernel(
    feature_s2t, feature_target, label_s2t, label_target,
    centroid_s2t, centroid_target,
):
    global _NC_CACHE
    if _NC_CACHE is None:
        _NC_CACHE = _build_nc()
    nc = _NC_CACHE

    # cross assignment: s2t features vs target centroids, and vice versa
    in_maps = _prep_domain(feature_s2t, centroid_target) + _prep_domain(
        feature_target, centroid_s2t
    )
    res = run_bass_kernel_spmd(nc, in_maps, core_ids=list(range(8))).results
    mask_s2t = np.stack([res[i]["mask"] for i in range(B)]).astype(np.int32)
    mask_target = np.stack([res[B + i]["mask"] for i in range(B)]).astype(
        np.int32
    )
    return (mask_s2t, mask_target)


# revision 16
# speedup vs baseline: 1.0937x; 1.0937x over previous
"""VQ codebook assignment + nearest upsample on 8 NeuronCores.

Problem (per domain): given features f [B=4, C=256, H=64, W=128] and
centroids c [K=19, C=256], compute argmin_k ||f[b,:,h,w] - c_k||^2 and
nearest-upsample the [64,128] index map to [512,1024] (8x per axis).
Two domains (cross-assigned centroids) x 4 batches = 8 cores, one
batch-image per core, no cross-core communication.

Design (fp16 matmul + int32 fixed-point scores + packed argmin);
measured ~38 us max-core / ~35.6 us mean vs the 64-76 us fp32 v1:

  * Features/centroids rounded to fp16 on the host: 1 cycle/row on the
    PE (fp32 is 4) and 4.2 MB/core of input DMA (half of fp32).
    Measured flip rate vs the fp32 reference: 0.04% of pixels ->
    rel_err 1.50e-2, under the 2e-2 gate (bf16 fails at 3.8e-2; the
    error is dominated by fp16 input rounding, deterministic for the
    fixed test inputs).
  * Centroids are pre-scaled by 256 (exact in fp16, a power of two), so
    fp32 PSUM scores are 256*(f.c_k). A bit-exact ScalarE Copy (the
    non-LUT Activation path) converts them to int32 score units; all
    downstream arithmetic is exact integer math. Score quantization
    error (1/512 in feature units) is ~3x below the fp16 input error.
  * -|c_k|^2/2 bias is folded into a host-built int32 "bias-iota"
    table: B = -32*score + (-32*bq_k + k), computed by one DVE
    scalar_tensor_tensor, then ONE min-reduce over k and (B & 31)
    recovers k. Ties pick the smaller k = jnp.argmin first-match
    semantics, exactly. Padding k's (19..31) get +2^30 so they never
    win. This replaces the 5-op max/is_ge/select/min argmax chain --
    DVE reduces and comparisons run at 1x (only some ALU ops get the
    2-byte 2x mode), so fewer + cheaper ops win.
  * The K-partition -> pixel-partition transpose that cost the fp32 v1
    64 PE passes is ONE DVE 32x32 StreamTranspose per 2048-px
    superblock, reading the packed int32 score tile. The host
    pre-permutes feature pixels into (sb, cch, h%16, w%32) tile order
    so the block-transposed layout lands directly as idxv[w, h]: the
    matmul for chunk cch writes PSUM partitions 32cch..+32 (PE PSUM
    writes only allow partition bases {0,32,64} -> two [64,512] PSUM
    tiles per superblock), and after the 32x32 block transpose
    partition = w, column-block = h. The last superblock's chain is
    column-split in two so the drain pipelines.
  * DMA: each HWDGE queue (SP/ACT) sustains ~150-190 GB/s only when
    pieces queue back-to-back with ~4KB-per-partition packets (the
    per-queue window is ~4 packets in flight at ~23 GB/s per DMA
    engine; the gpsimd SWDGE queue dispatches ~4x slower, ~24 GB/s, and
    only carries the tiny bias table). Input pieces arrive in
    superblock order on alternating queues (~290 GB/s aggregate,
    ~14 us); sb3 is split so the last piece is small.
  * Upsample tail per h-half: DVE 32x32 transpose + block copies build
    idxT[h, w], a broadcast copy replicates 8x in x converting to int8
    (split DVE+ACT mid-kernel, DVE-only for the drain-critical half),
    and stores write [*, 8y, 1024] with a stride-0 source loop for the
    8x y-replication (1KB runs; the final store splits across both
    HWDGE queues). int8 mask out (0.5 MB); the host upcasts to int32.

Remaining wall-clock anatomy (per core): ~8 us fixed prologue (runtime
handshake + DGE ring config + barriers), ~14 us input DMA, ~8 us
drain (sem-prop + last superblock compute + replicate + store), ~3 us
epilogue. PE ~10 us and DVE ~12 us active both hide under the DMA.
"""

import numpy as np

import concourse.bass as bass
import concourse.mybir as mybir
import concourse.tile as tile
from concourse import bacc
from concourse.bass import ds
from concourse.bass_utils import run_bass_kernel_spmd

F32 = mybir.dt.float32
F16 = mybir.dt.float16
I32 = mybir.dt.int32
I16 = mybir.dt.int16
I8 = mybir.dt.int8

B = 4
C = 256
H, W = 64, 128
K = 19
KP = 32               # K padded to a 32x32 transpose block
HL, WL = 512, 1024
NPIX = H * W          # 8192
SB = 4                # superblocks (2048 px each)
SBPIX = NPIX // SB
CH = 512              # matmul moving chunk (pixels)
NCH = SBPIX // CH     # chunks per superblock: 4
NJ = CH // KP         # 32-col blocks per score tile: 16
UP = HL // H          # 8x upsample
SC = 256.0            # centroid pre-scale -> int16 score units
FWC = KP + NPIX       # fw columns: [w | pixels]

_NC_CACHE = None


def _build_nc():
    nc = bacc.Bacc("TRN2", target_bir_lowering=False, debug=False)

    fw_in = nc.dram_tensor("fw", [C, FWC], F16, kind="ExternalInput")
    bi_in = nc.dram_tensor("biasiota", [128, KP], I32, kind="ExternalInput")
    mask_out = nc.dram_tensor("mask", [HL, WL], I8, kind="ExternalOutput")

    fwv = fw_in.ap().rearrange("(a p) n -> a p n", a=2)       # [2, 128, FWC]
    outv = mask_out.ap().rearrange("(h y) x -> h y x", y=UP)  # [64, 8, 1024]

    with tile.TileContext(nc) as tc:
        with (
            tc.tile_pool(name="persist", bufs=1) as pp,
            tc.tile_pool(name="work", bufs=2) as wp,
            tc.tile_pool(name="psA", bufs=3, space="PSUM") as psA,
        ):
            fw0 = pp.tile([128, FWC], F16, tag="fw0")
            fw1 = pp.tile([128, FWC], F16, tag="fw1")
            bi32 = pp.tile([128, KP], I32, tag="bi32")
            idxv = pp.tile([128, H], I32, tag="idxv")       # [w, h]
            tmp16 = pp.tile([128, H], I32, tag="tmp16")     # block-transposed
            idxT = pp.tile([H, W], I32, tag="idxT")         # [h, w]
            rep = pp.tile([H, WL], I8, tag="rep")           # x-replicated

            # --- input loads. Both HWDGE queues (SP/ACT) sustain
            # ~150-190 GB/s each when pieces queue back-to-back (the SWDGE
            # gpsimd queue only manages ~24 GB/s -- it gets just the tiny
            # bias load). Pieces arrive in superblock order on alternating
            # queues; sb3 is split so its last piece is small (short
            # drain). ---
            nc.gpsimd.dma_start(bi32, bi_in[:, :])
            pieces = [
                ds(0, KP + SBPIX),                           # w + sb0
                ds(KP + SBPIX, SBPIX),                       # sb1
                ds(KP + 2 * SBPIX, SBPIX),                   # sb2
                ds(KP + 3 * SBPIX, 3 * SBPIX // 4),          # sb3 front
                ds(KP + 3 * SBPIX + 3 * SBPIX // 4, SBPIX // 4),  # sb3 tail
            ]
            for pi, sl in enumerate(pieces):
                for half in range(2):
                    dst = fw0 if half == 0 else fw1
                    eng = nc.sync if (pi + half) % 2 == 0 else nc.scalar
                    eng.dma_start(dst[:, sl], fwv[half, :, sl])

            bi_b = bi32.rearrange("p (o k) -> p o k", o=1).to_broadcast(
                [128, NJ, KP]
            )

            # --- per-superblock: 8 matmuls -> int16 scores -> 32x32 block
            # transpose -> packed argmin over k ---
            for sb in range(SB):
                psa = psA.tile([64, CH], F32, tag="psa")
                psb = psA.tile([64, CH], F32, tag="psb")
                pst = [psa, psb]
                for cch in range(NCH):
                    colsl = ds(KP + sb * SBPIX + cch * CH, CH)
                    ps = pst[cch // 2]
                    psl = ds(32 * (cch % 2), 32)
                    nc.tensor.matmul(
                        ps[psl, :], fw0[:, 0:KP], fw0[:, colsl],
                        start=True, stop=False,
                    )
                    nc.tensor.matmul(
                        ps[psl, :], fw1[:, 0:KP], fw1[:, colsl],
                        start=False, stop=True,
                    )
                # bit-exact ScalarE Copy: fp32 PSUM -> int32 (RNE)
                St = wp.tile([128, CH], I32, tag="St")
                nc.scalar.copy(St[ds(0, 64), :], pst[0])
                nc.scalar.copy(St[ds(64, 64), :], pst[1])
                # DVE 32x32 block transpose -> partition=w, col-block=h,
                # then B = -32*score + (-32*bq_k + k); min over k; k = B&31.
                # The last superblock is column-split in two so its chain
                # pipelines during the drain.
                T = wp.tile([128, CH], I32, tag="T")
                Bt = wp.tile([128, CH], I32, tag="Bt")
                Bm = wp.tile([128, NJ], I32, tag="Bm")
                nsp = 2 if sb == SB - 1 else 1
                cw = CH // nsp
                for cs in range(nsp):
                    csl = ds(cs * cw, cw)
                    nc.vector.transpose(T[:, csl], St[:, csl])
                    nc.vector.scalar_tensor_tensor(
                        Bt[:, csl].rearrange("p (j k) -> p j k", k=KP),
                        T[:, csl].rearrange("p (j k) -> p j k", k=KP),
                        -32, bi32.rearrange("p (o k) -> p o k", o=1)
                        .to_broadcast([128, cw // KP, KP]),
                        op0=mybir.AluOpType.mult, op1=mybir.AluOpType.add,
                    )
                    bsl = ds(cs * (cw // KP), cw // KP)
                    nc.vector.tensor_reduce(
                        Bm[:, bsl],
                        Bt[:, csl].rearrange("p (j k) -> p j k", k=KP),
                        axis=mybir.AxisListType.X, op=mybir.AluOpType.min,
                    )
                    nc.vector.tensor_scalar(
                        idxv[:, ds(sb * NJ + cs * (cw // KP), cw // KP)],
                        Bm[:, bsl], 31, None,
                        op0=mybir.AluOpType.bitwise_and,
                    )

                # --- tail, overlapped: after each half of the superblocks,
                # emit that h-half (transpose, x8-replicate, store) ---
                if sb % (SB // 2) != SB // 2 - 1:
                    continue
                hh = sb // (SB // 2)           # 0 or 1
                hsl = ds(hh * H // 2, H // 2)  # 32 h columns
                psl = ds(hh * 32, 32)          # matching partition rows
                # hh0's tail is ready mid-kernel but the static scheduler
                # bakes its DVE ops into the drain-critical sequence; keep
                # everything after its transpose OFF the DVE stream (ACT
                # has mid-kernel slack). hh1's tail stays all-DVE.
                nc.vector.transpose(tmp16[:, hsl], idxv[:, hsl])
                for i in range(W // 32):
                    if hh == 0:
                        nc.scalar.copy(
                            idxT[psl, ds(32 * i, 32)],
                            tmp16[ds(32 * i, 32), hsl],
                        )
                    else:
                        nc.vector.tensor_copy(
                            idxT[psl, ds(32 * i, 32)],
                            tmp16[ds(32 * i, 32), hsl],
                        )
                # 8x replicate along x, int8 out. Mid-kernel half splits
                # across DVE+ScalarE; the final half runs entirely on DVE
                # (ScalarE showed multi-us scheduling gaps at the drain).
                repv = rep[psl].rearrange("p (w x) -> p w x", w=W)
                if hh == 0:
                    idxT_b = idxT[psl].rearrange(
                        "p (w o) -> p w o", o=1
                    ).to_broadcast([32, W, UP])
                    nc.scalar.copy(repv, idxT_b)
                else:
                    for wh in range(2):
                        idxT_b = idxT[psl, ds(wh * W // 2, W // 2)].rearrange(
                            "p (w o) -> p w o", o=1
                        ).to_broadcast([32, W // 2, UP])
                        half = repv[:, ds(wh * W // 2, W // 2)]
                        nc.vector.tensor_copy(half, idxT_b)
                # single store-DMA per half; stride-0 source loop re-reads
                # each 1KB SBUF row 8x for the y-replication
                # stores: hh0 whole on SP (overlaps compute); the final
                # one splits 2x2 across both HWDGE queues so each queue
                # pipelines two entries (shorter drain)
                if hh == 0:
                    splits = ((nc.sync, 0, 32),)
                else:
                    splits = (
                        (nc.sync, 0, 8), (nc.scalar, 8, 8),
                        (nc.sync, 16, 8), (nc.scalar, 24, 8),
                    )
                for eng, p0, np_ in splits:
                    pssl = ds(hh * 32 + p0, np_)
                    srcap = rep[pssl].rearrange(
                        "p (o x) -> p o x", o=1
                    ).to_broadcast([np_, UP, WL])
                    eng.dma_start(outv[pssl], srcap)

    nc.compile()
    return nc


def _prep_domain(feature, centroid):
    """Per-core inputs for one domain: 4 batches against one centroid set."""
    c = np.asarray(centroid, dtype=np.float64)                  # [K, C]
    w16 = c.T.astype(np.float16)                                # [C, K]
    wsc = (w16.astype(np.float32) * SC).astype(np.float16)      # exact x2^8
    wpad = np.zeros((C, KP), dtype=np.float16)
    wpad[:, :K] = wsc
    c2 = np.sum(c * c, axis=1)                                  # [K]
    bq = np.rint(SC * (c2.mean() - c2) / 2.0).astype(np.int64)
    biasiota = np.full(KP, 2**30, dtype=np.int64)
    biasiota[:K] = -32 * bq + np.arange(K)
    biasiota = np.ascontiguousarray(
        np.tile(biasiota[None, :], (128, 1)), dtype=np.int32
    )
    maps = []
    for b in range(B):
        f16 = np.asarray(feature[b], dtype=np.float32).astype(np.float16)
        # pixel permutation: image (h, w) -> chunk order (sb, cch, h%16, w%32)
        fp = (
            f16.reshape(C, SB, 16, W // 32, 32)
            .transpose(0, 1, 3, 2, 4)
            .reshape(C, NPIX)
        )
        fw = np.ascontiguousarray(np.concatenate([wpad, fp], axis=1))
        maps.append({"fw": fw, "biasiota": biasiota})
    return maps


def kernel(
    feature_s2t, feature_target, label_s2t, label_target,
    centroid_s2t, centroid_target,
):
    global _NC_CACHE
    if _NC_CACHE is None:
        _NC_CACHE = _build_nc()
    nc = _NC_CACHE

    # cross assignment: s2t features vs target centroids, and vice versa
    in_maps = _prep_domain(feature_s2t, centroid_target) + _prep_domain(
        feature_target, centroid_s2t
    )
    res = run_bass_kernel_spmd(nc, in_maps, core_ids=list(range(8))).results
    mask_s2t = np.stack([res[i]["mask"] for i in range(B)]).astype(np.int32)
    mask_target = np.stack([res[B + i]["mask"] for i in range(B)]).astype(
        np.int32
    )
    return (mask_s2t, mask_target)
